# revision 1
# baseline (speedup 1.0000x reference)
"""Min-plus matmul, v4: instruction-count-minimal brute force.

Per-instruction overhead on this stack is ~60us, so the design uses as few,
as large, DVE instructions as possible:
  - shard over out_features: core k owns o in [128k, 128(k+1)); W shard
    [128, 1024] sits on partitions (one load, no broadcasts of W needed).
  - batch G=32 batch-rows per group: one DMA broadcasts x[32g:32g+32, :]
    across 128 partitions ([128, 32, 1024], 16MB); one tensor_tensor add
    with W repeated via a stride-0 middle AP dim (in-place over the x
    broadcast buffer); one 3D tensor_reduce(min) over i -> [128, 32]
    columns of ost.
  - 16 groups x ~3 instructions per core per pass.
"""

from contextlib import ExitStack

import numpy as np

import concourse.bass as bass
import concourse.mybir as mybir
from concourse.bass_utils import run_bass_kernel_spmd

B, OUT, IN = 512, 1024, 1024
NCORES = 8
OSH = OUT // NCORES  # 128
G = 47  # max batch rows per group (SBUF-bound: 47*4KB + 4KB wt = 192KB/part)
# non-uniform groups: 10x47 + 1x42 = 512 rows, 11 groups
GROUPS = [(s, min(G, B - s)) for s in range(0, B, G)]
F32 = mybir.dt.float32
AL = mybir.AluOpType


def _build_program(repeat: int = 1):
    nc = bass.Bass("TRN2", target_bir_lowering=False, debug=False)
    x = nc.dram_tensor("x", [B, IN], F32, kind="ExternalInput").ap()
    w = nc.dram_tensor("w", [OSH, IN], F32, kind="ExternalInput").ap()
    out = nc.dram_tensor("out", [OSH, B], F32, kind="ExternalOutput").ap()

    with ExitStack() as ctx:
        wt = ctx.enter_context(nc.sbuf_tensor("wt", [128, IN], F32))
        xb = ctx.enter_context(nc.sbuf_tensor("xb", [128, G * IN], F32))
        ost = ctx.enter_context(nc.sbuf_tensor("ost", [128, B], F32))

        wsem = ctx.enter_context(nc.semaphore())
        bsem = ctx.enter_context(nc.semaphore())
        rsem = ctx.enter_context(nc.semaphore())
        osem = ctx.enter_context(nc.semaphore())
        block = ctx.enter_context(nc.Block())

        @block.sync
        def _(sync):
            # W load counts into bsem so the first TT's single attached wait
            # covers both W and the first broadcast (one wait per instr max).
            sync.dma_start(wt[:], w[:, :]).then_inc(bsem, 16)
            for n in range(repeat):
                for g, (s, gl) in enumerate(GROUPS):
                    t = n * len(GROUPS) + g
                    src = x[s : s + gl, :]
                    bc = bass.AP(src.tensor, src.offset, [[0, 128]] + src.ap)
                    ins = sync.dma_start(xb[:, : gl * IN], bc)
                    if t >= 1:
                        # single xb buffer: previous group's reduce done
                        ins._wait_ge(rsem, t)
                    ins.then_inc(bsem, 16)
                sync.dma_start(out[:, :], ost[:])._wait_ge(
                    rsem, len(GROUPS) * (n + 1)
                ).then_inc(osem, 16)

        @block.vector
        def _(vector):
            for n in range(repeat):
                for g, (s, gl) in enumerate(GROUPS):
                    t = n * len(GROUPS) + g
                    x3 = xb[:, : gl * IN].rearrange(
                        "p (g i) -> p g i", g=gl
                    )
                    wrep = bass.AP(
                        wt[:].tensor,
                        wt[:].offset,
                        [wt[:].ap[0], [0, gl], wt[:].ap[1]],
                    )
                    nc.vector.tensor_tensor(
                        out=x3, in0=wrep, in1=x3, op=AL.add
                    )._wait_ge(bsem, 16 * (t + 2))
                    nc.vector.tensor_reduce(
                        out=ost[:, s : s + gl],
                        in_=x3,
                        axis=mybir.AxisListType.X,
                        op=AL.min,
                    ).then_inc(rsem, 1)

    return nc


def _prep_host(x, W):
    return [
        {"x": x, "w": np.ascontiguousarray(W[OSH * k : OSH * (k + 1), :])}
        for k in range(NCORES)
    ]


def kernel(x: np.ndarray, W: np.ndarray) -> np.ndarray:
    x = np.ascontiguousarray(np.asarray(x, dtype=np.float32))
    W = np.ascontiguousarray(np.asarray(W, dtype=np.float32))
    assert x.shape == (B, IN) and W.shape == (OUT, IN)

    nc = _build_program()
    in_maps = _prep_host(x, W)
    res = run_bass_kernel_spmd(nc, in_maps, core_ids=list(range(NCORES)))
    # out dram [OSH, B] per core: out[o_local, b] -> full[b, OSH*k + o_local]
    full = np.empty((B, OUT), dtype=np.float32)
    for k in range(NCORES):
        full[:, OSH * k : OSH * (k + 1)] = res.results[k]["out"].T
    return full

